# revision 24
# baseline (speedup 1.0000x reference)
"""Trainium2 Bass kernel for CapsNet dynamic routing (ClassCapsules).

Reference (B=256, R=1152, C=10, O=16, I=8, 3 routing iters):
    u_hat[b,r,c,o] = sum_i W[r,c,o,i] * x[b,r,i]
    b_ij = 0
    for it in 3:
        c_ij = softmax(b_ij, axis=1)                      # over c
        s = sum_r c_ij[r,c] * u_hat[b,r,c,o] + bias       # [B,C,O]
        v = squash(s)
        if it < 2:
            b_ij += mean_b sum_o u_hat[b,r,c,o] v[b,c,o]  # [R,C]
    return v[..., None]

u_hat (189MB) is never materialized; both contractions are re-associated as
    s[b,(co)]  = x~[b,(ri)] @ (c o W~)[(ri),(co)]
    agree[r,c] = sum_{i,o} W~ o G,  G = (1/B) x~^T v.

Distribution: collective-free full replication (CC init on this part costs
55-134us + ~10us per AllReduce, far more than recomputing).  Every core
computes full-batch routing state; the final iteration computes only the
core's own 32-batch output shard.

Implementation notes (HW-calibrated):
  * All big matmuls are MIXED fp8e4(stationary x) x fp16(moving W/CW/v):
    at full PE p-state the pair cadence is ~70ns (stream-bound, LDWEIGHTS
    fully hidden).  fp8 is ONLY on x and only for the routing iterations;
    the final iteration's s uses fp16 x, keeping output error ~1e-3.
  * o-major column order (co) = (o,c): every large DVE op has a packed
    fp16 last dim -> 2x DVE rate.  Broadcasts (c_ij over o, fac over o)
    sit on 0-stride OUTER dims which keep 2x eligibility.
  * bias is folded into each s accumulation as a 73rd matmul (ones/128
    stationary, broadcast bias moving), so PSUM evacuation is a single
    ACT scale+cast copy instead of DVE STT ops.
  * ACT (otherwise idle) does all PSUM evacuation, Square/Sqrt/Exp;
    GPSIMD only issues the xb8/xo16 DMAs (second DMA queue).
  * The agree->softmax->c_ij->CW->next-s chain is processed per HALF
    (36 groups): the next iteration's s accumulation starts while the
    current agree stage is still finishing the other half.
"""

import os
import sys
import types

sys.path.insert(0, "/opt/trn_rl_repo")

# Shim antenv.axon_hooks (absent on this image) so BASS_TRACE=1 profiling
# works through run_bass_kernel_spmd's axon path.  Harmless when unused.
try:
    import antenv.axon_hooks  # noqa: F401
except ImportError:
    try:
        _hooks = types.ModuleType("antenv.axon_hooks")
        _hooks._hook = None
        _hooks.set_axon_ntff_profile_hook = lambda h: setattr(_hooks, "_hook", h)
        _hooks.get_axon_ntff_profile_hook = lambda: _hooks._hook
        sys.modules["antenv.axon_hooks"] = _hooks
        import antenv
        antenv.axon_hooks = _hooks
        from trn_agent_boot.trn_boot import _ntff_profile_via_ctypes
        _hooks.set_axon_ntff_profile_hook(
            _ntff_profile_via_ctypes("/opt/axon/libaxon_pjrt.so")
        )
    except Exception:
        pass

import numpy as np
import ml_dtypes

import concourse.bacc as bacc
import concourse.bass as bass
import concourse.tile as tile
from concourse import mybir
import concourse.bass_utils as _bass_utils
from concourse.bass_utils import run_bass_kernel_spmd

if os.environ.get("BASS_TRACE"):
    _bass_utils.upload_artifacts = lambda tmpdir: ""  # no bucket access here

LAST_RESULT = None

F32 = mybir.dt.float32
F16 = mybir.dt.float16
F8 = mybir.dt.float8e4
ALU = mybir.AluOpType
ACT = mybir.ActivationFunctionType

B, R, C, O, I = 256, 1152, 10, 16, 8
CO = C * O                      # 160
N_CORES = 8
RI = R * I                      # 9216
NG = RI // 128                  # 72 groups of 128 (r,i) rows
GG = 9                          # groups per chunk
NCH = NG // GG                  # 8 chunks
NHC = NCH // 2                  # chunks per half
HG = NG // 2                    # groups per half (36)
NB = B // 128                   # 2 batch partition blocks
B_SHARD = B // N_CORES          # 32
RPG = 128 // I                  # 16 r-slots per partition group
SX = 8.0                        # x fp8 pre-scale
SW = 64.0                       # W fp8 pre-scale (it0 rhs only)
SG = 64.0 / (SX * B)            # G psum -> fp16 scale (1/32)
ISUM = 1.0 / 64.0               # sel entries: undo the 64, fold mean_b
ITERS = 3


def build():
    nc = bacc.Bacc("TRN2", target_bir_lowering=False, debug=False,
                   num_devices=N_CORES)

    xt8_d = nc.dram_tensor("xt8", [128, NG, B], F8, kind="ExternalInput")
    w8_d = nc.dram_tensor("w8", [128, NG, CO], F8, kind="ExternalInput")
    xb8_d = nc.dram_tensor("xb8", [128, NG, NB, 128], F8, kind="ExternalInput")
    xo16_d = nc.dram_tensor("xo16", [128, NG, B_SHARD], F16,
                            kind="ExternalInput")
    w16_d = nc.dram_tensor("w16", [128, NG, CO], F16, kind="ExternalInput")
    bias_d = nc.dram_tensor("biasf", [ITERS, CO], F16, kind="ExternalInput")
    sel_d = nc.dram_tensor("sel", [128, RPG], F16, kind="ExternalInput")
    selT_d = nc.dram_tensor("selT", [RPG, 128], F16, kind="ExternalInput")
    y_d = nc.dram_tensor("y", [B_SHARD, CO], F32, kind="ExternalOutput")

    with tile.TileContext(nc) as tc:
        with (
            tc.tile_pool(name="singles", bufs=1) as singles,
            tc.tile_pool(name="work", bufs=3) as work,
            tc.tile_pool(name="small", bufs=2) as small,
            tc.tile_pool(name="psum_s", bufs=1, space="PSUM") as psum_s,
            tc.tile_pool(name="psum_g", bufs=3, space="PSUM") as psum_g,
            tc.tile_pool(name="psum_m", bufs=2, space="PSUM") as psum_m,
        ):
            # ---- ACT table preloads (overlap the DMA wait) ----
            warm = singles.tile([128, 2], F32)
            nc.vector.memset(warm, 1.0)
            warm2 = singles.tile([128, 2], F32)
            nc.scalar.activation(warm2, warm, ACT.Exp)
            nc.scalar.activation(warm2, warm, ACT.Sqrt)
            nc.scalar.activation(warm2, warm, ACT.Square)
            nc.scalar.copy(warm2, warm)

            eps_sb = singles.tile([128, 1], F32)
            nc.vector.memset(eps_sb, 1e-8)
            ones16 = singles.tile([128, 128], F16)
            nc.vector.memset(ones16, 1.0 / 128.0)

            # ---- bulk loads, all on the sync queue (saturates per-core
            # DMA bw; a second queue only contends).  s0 needs just
            # w8(fp8)+xt8 = 3.8MB; xb8/w16 follow, paced for the G stage.
            W8, W16, XT8, XB8 = [], [], [], []
            biasb = sel_sb = selT_sb = None
            for ch in range(NCH):
                w8_t = singles.tile([128, GG, CO], F8, tag=f"w8{ch}",
                                    name=f"w8_{ch}")
                nc.sync.dma_start(out=w8_t,
                                  in_=w8_d[:, ch * GG:(ch + 1) * GG, :])
                W8.append(w8_t)
                x_t = singles.tile([128, GG, B], F8, tag=f"xt{ch}",
                                   name=f"xt8_{ch}")
                nc.sync.dma_start(out=x_t,
                                  in_=xt8_d[:, ch * GG:(ch + 1) * GG, :])
                XT8.append(x_t)
                if ch == 0:
                    # small constants after the s0-critical first chunk
                    biasb = singles.tile([128, ITERS, CO], F16)
                    nc.sync.dma_start(
                        out=biasb,
                        in_=bass.AP(tensor=bias_d, offset=0,
                                    ap=[[0, 128], [1, ITERS * CO]]),
                    )
                    sel_sb = singles.tile([128, RPG], F16)
                    nc.sync.dma_start(out=sel_sb, in_=sel_d[:, :])
                    selT_sb = singles.tile([RPG, 128], F16)
                    nc.sync.dma_start(out=selT_sb, in_=selT_d[:, :])
            for ch in range(NCH):
                b_t = singles.tile([128, GG, NB, 128], F8, tag=f"xb{ch}",
                                   name=f"xb8_{ch}")
                nc.sync.dma_start(out=b_t,
                                  in_=xb8_d[:, ch * GG:(ch + 1) * GG, :, :])
                XB8.append(b_t)
                w_t = singles.tile([128, GG, CO], F16, tag=f"w{ch}",
                                   name=f"w16_{ch}")
                nc.sync.dma_start(out=w_t,
                                  in_=w16_d[:, ch * GG:(ch + 1) * GG, :])
                W16.append(w_t)
            XO16 = singles.tile([128, NG, B_SHARD], F16)
            nc.sync.dma_start(out=XO16, in_=xo16_d[:, :, :])

            # ---------------- helpers ----------------
            def s_mms(it, ch, rhs_t, s_ps, kbs=None):
                """Issue the s-accumulation matmuls for one 9-group chunk."""
                final = it == ITERS - 1
                for gg in range(GG):
                    g = ch * GG + gg
                    if not final:
                        for kb in (range(NB) if kbs is None else kbs):
                            nc.tensor.matmul(
                                s_ps[kb],
                                XT8[ch][:, gg, kb * 128:(kb + 1) * 128],
                                rhs_t[:, gg, :],
                                start=(g == 0), stop=False,
                            )
                    else:
                        nc.tensor.matmul(
                            s_ps[0][:B_SHARD, :],
                            XO16[:, g, :],
                            rhs_t[:, gg, :],
                            start=(g == 0), stop=False,
                        )

            def s_bias(it, s_ps, kb=None):
                final = it == ITERS - 1
                bias_mv = biasb[:, it, :]
                if not final:
                    for k in (range(NB) if kb is None else [kb]):
                        nc.tensor.matmul(s_ps[k], ones16, bias_mv,
                                         start=False, stop=True)
                else:
                    nc.tensor.matmul(s_ps[0][:B_SHARD, :],
                                     ones16[:, :B_SHARD], bias_mv,
                                     start=False, stop=True)

            def squash(it, s_ps, v_t, tagx=""):
                """v = squash(s_psum*scal); bias already in psum (scaled)."""
                final = it == ITERS - 1
                nparts = B_SHARD if final else 128
                nb = len(s_ps)
                scal = (0.1 / (SX * SW), 1.0 / SX, 1.0)[it]
                t = work.tile([nparts, nb, CO], F16, tag="t" + tagx,
                              name=f"t_{it}{tagx}")
                sq = work.tile([nparts, nb, CO], F16, tag="sq" + tagx,
                               name=f"sq_{it}{tagx}")
                for kb in range(nb):
                    nc.vector.tensor_scalar_mul(t[:, kb, :], s_ps[kb], scal)
                nc.vector.tensor_mul(sq, t, t)
                n2 = work.tile([nparts, nb, C], F32, tag="n2",
                               name=f"n2_{it}")
                nc.vector.reduce_sum(
                    n2, sq.rearrange("p nb (o c) -> p nb c o", o=O),
                    axis=mybir.AxisListType.X,
                )
                n2f = n2.rearrange("p nb c -> p (nb c)")
                rt = work.tile([nparts, nb * C], F32, tag="rt",
                               name=f"rt_{it}")
                nc.scalar.activation(rt, n2f, ACT.Sqrt, bias=eps_sb[:nparts])
                den = work.tile([nparts, nb * C], F32, tag="den",
                                name=f"den_{it}")
                nc.vector.scalar_tensor_tensor(
                    out=den, in0=n2f, scalar=1.0, in1=rt, op0=ALU.add,
                    op1=ALU.mult,
                )
                rec = work.tile([nparts, nb * C], F32, tag="rec",
                                name=f"rec_{it}")
                nc.vector.reciprocal(rec, den)
                fac = work.tile([nparts, nb * C], F16, tag="fac",
                                name=f"fac_{it}")
                nc.vector.tensor_mul(fac, n2f, rec)
                fac_b = fac.rearrange("p (nb c) -> p nb c", nb=nb).rearrange(
                    "p nb (one c) -> p nb one c", one=1).broadcast_to(
                    [nparts, nb, O, C])
                nc.vector.tensor_tensor(
                    out=v_t.rearrange("p nb (o c) -> p nb o c", o=O),
                    in0=t.rearrange("p nb (o c) -> p nb o c", o=O),
                    in1=fac_b, op=ALU.mult,
                )

            # ================= iteration 0: s from W directly =============
            s_ps = [psum_s.tile([128, CO], F32, tag=f"s{kb}",
                                name=f"s_ps{kb}_0") for kb in range(NB)]
            v16 = work.tile([128, NB, CO], F16, tag="v", name="v_0")
            for ch in range(NCH):
                s_mms(0, ch, W8[ch], s_ps, kbs=[0])
            s_bias(0, s_ps, kb=0)
            for ch in range(NCH):
                s_mms(0, ch, W8[ch], s_ps, kbs=[1])
            squash(0, [s_ps[0]], v16[:, 0:1, :], tagx="a")
            s_bias(0, s_ps, kb=1)
            squash(0, [s_ps[1]], v16[:, 1:2, :], tagx="b")

            # ============ G/agree stages with embedded next-s =============
            esr = None
            for it in range(ITERS - 1):
                nxt = it + 1
                final_next = nxt == ITERS - 1
                q16 = small.tile([128, NG, C], F16, tag="q", name=f"q_{it}")
                esr_prev = esr
                esr = small.tile([RPG, NG * C + NG], F16, tag="esr",
                                 name=f"esr_{it}")
                ee16 = small.tile([128, NG, C], F16, tag="ee",
                                  name=f"ee_{it}")
                if final_next:
                    s_ps = [psum_s.tile([128, CO], F32, tag="s0",
                                        name="s2_ps")]
                else:
                    s_ps = [psum_s.tile([128, CO], F32, tag=f"s{kb}",
                                        name=f"s_ps{kb}_{nxt}")
                            for kb in range(NB)]

                cw_tiles = {}

                def softmax_and_next_s(h):
                    lo, hi = h * HG * C, (h + 1) * HG * C
                    agree_ps = psum_m.tile([128, 400], F32, tag="m",
                                           name=f"agree_{it}_{h}")
                    nc.tensor.matmul(
                        agree_ps[:RPG, :HG * C], sel_sb,
                        q16.rearrange("p g c -> p (g c)")[:, lo:hi],
                        start=True, stop=True)
                    if it == 0:
                        nc.scalar.activation(esr[:, lo:hi],
                                             agree_ps[:RPG, :HG * C],
                                             ACT.Exp)
                    else:
                        ex = small.tile([RPG, HG * C], F16, tag="ex",
                                        name=f"ex_{it}_{h}")
                        nc.scalar.activation(ex, agree_ps[:RPG, :HG * C],
                                             ACT.Exp)
                        nc.vector.tensor_mul(esr[:, lo:hi],
                                             esr_prev[:, lo:hi], ex)
                    dlo = NG * C + h * HG
                    den = small.tile([RPG, HG], F32, tag="sden",
                                     name=f"den_{it}_{h}")
                    nc.vector.reduce_sum(
                        den,
                        esr[:, lo:hi].rearrange("p (g c) -> p g c", g=HG),
                        axis=mybir.AxisListType.X,
                    )
                    with nc.allow_low_precision(reason="softmax recip"):
                        nc.vector.reciprocal(esr[:, dlo:dlo + HG], den)
                    # ---- broadcast esr to all partitions (PE) ----
                    cp16 = small.tile([128, HG * C + HG], F16, tag="cp",
                                      name=f"cp_{it}_{h}")
                    cp_ps = psum_m.tile([128, 400], F32, tag="m",
                                        name=f"cpA_{it}_{h}")
                    nc.tensor.matmul(cp_ps[:, :HG * C], selT_sb,
                                     esr[:, lo:hi], start=True, stop=True)
                    nc.scalar.copy(cp16[:, :HG * C], cp_ps[:, :HG * C])
                    cp_ps2 = psum_m.tile([128, 400], F32, tag="m",
                                         name=f"cpB_{it}_{h}")
                    nc.tensor.matmul(cp_ps2[:, :HG], selT_sb,
                                     esr[:, dlo:dlo + HG], start=True,
                                     stop=True)
                    nc.scalar.copy(cp16[:, HG * C:], cp_ps2[:, :HG])
                    rec_b = cp16[:, HG * C:].rearrange(
                        "p (g one) -> p g one", one=1).broadcast_to(
                        [128, HG, C])
                    nc.vector.tensor_tensor(
                        out=ee16[:, h * HG:(h + 1) * HG, :],
                        in0=cp16[:, :HG * C].rearrange("p (g c) -> p g c",
                                                       g=HG),
                        in1=rec_b, op=ALU.mult,
                    )
                    # ---- CW for this half (DVE), prioritized ----
                    for ch in range(h * NHC, (h + 1) * NHC):
                        cw = work.tile([128, GG, CO], F16, tag="cw",
                                       name=f"cw_{nxt}_{ch}")
                        ee_b = ee16[:, ch * GG:(ch + 1) * GG, :].rearrange(
                            "p g (one c) -> p g one c", one=1).broadcast_to(
                            [128, GG, O, C])
                        nc.vector.tensor_tensor(
                            out=cw.rearrange("p g (o c) -> p g o c", o=O),
                            in0=W16[ch].rearrange("p g (o c) -> p g o c",
                                                  o=O),
                            in1=ee_b, op=ALU.mult,
                        )
                        cw_tiles[ch] = cw

                def next_s_mms(h):
                    for ch in range(h * NHC, (h + 1) * NHC):
                        if final_next:
                            s_mms(nxt, ch, cw_tiles[ch], s_ps)
                        else:
                            s_mms(nxt, ch, cw_tiles[ch], s_ps, kbs=[0])

                for ch in range(NCH):
                    # ---- G matmuls + W o G + Q for this chunk ----
                    g16 = work.tile([128, GG, CO], F16, tag="g16",
                                    name=f"g16_{it}_{ch}")
                    for t3 in range(3):
                        g_ps = psum_g.tile([128, 3, CO], F32, tag="g",
                                           name=f"g_ps_{it}_{ch}_{t3}")
                        for j in range(3):
                            for kb in range(NB):
                                nc.tensor.matmul(
                                    g_ps[:, j, :],
                                    XB8[ch][:, t3 * 3 + j, kb, :],
                                    v16[:, kb, :],
                                    start=(kb == 0),
                                    stop=(kb == NB - 1),
                                )
                        nc.scalar.activation(
                            g16[:, t3 * 3:t3 * 3 + 3, :],
                            g_ps.rearrange("p a co -> p (a co)"),
                            ACT.Copy, scale=SG)
                    p16 = work.tile([128, GG, CO], F16, tag="p16",
                                    name=f"p16_{it}_{ch}")
                    nc.vector.tensor_mul(p16, W16[ch], g16)
                    if ch % 2 == 0:
                        h8 = work.tile([128, 2 * GG, O // 2, C], F16,
                                       tag="h8", name=f"h8_{it}_{ch}")
                    pv = p16.rearrange("p g (o c) -> p g o c", o=O)
                    hslice = h8[:, (ch % 2) * GG:(ch % 2 + 1) * GG, :, :]
                    nc.vector.tensor_tensor(
                        out=hslice, in0=pv[:, :, 0:O // 2, :],
                        in1=pv[:, :, O // 2:O, :], op=ALU.add)
                    if ch % 2 == 1:
                        h4 = work.tile([128, 2 * GG, O // 4, C], F16,
                                       tag="h4", name=f"h4_{it}_{ch}")
                        nc.vector.tensor_tensor(
                            out=h4, in0=h8[:, :, 0:O // 4, :],
                            in1=h8[:, :, O // 4:O // 2, :], op=ALU.add)
                        with nc.allow_low_precision(reason="Q fp16"):
                            nc.vector.reduce_sum(
                                q16[:, (ch - 1) * GG:(ch + 1) * GG, :],
                                h4.rearrange("p g o c -> p g c o"),
                                axis=mybir.AxisListType.X,
                            )
                    if ch == NCH - 3:
                        softmax_and_next_s(0)
                next_s_mms(0)
                softmax_and_next_s(1)
                next_s_mms(1)
                if final_next:
                    s_bias(nxt, s_ps)
                    yv = work.tile([B_SHARD, 1, CO], F32, tag="yv")
                    squash(nxt, [s_ps[0][:B_SHARD, :]], yv)
                    nc.sync.dma_start(
                        out=y_d[:, :],
                        in_=yv.rearrange("p one co -> p (one co)"))
                else:
                    s_bias(nxt, s_ps, kb=0)
                    for ch in range(NCH):
                        s_mms(nxt, ch, cw_tiles[ch], s_ps, kbs=[1])
                    v16 = work.tile([128, NB, CO], F16, tag="v",
                                    name=f"v_{nxt}")
                    squash(nxt, [s_ps[0]], v16[:, 0:1, :], tagx="a")
                    s_bias(nxt, s_ps, kb=1)
                    squash(nxt, [s_ps[1]], v16[:, 1:2, :], tagx="b")

    nc.compile()
    return nc


_NC = None


def kernel(x: np.ndarray, W: np.ndarray, bias: np.ndarray) -> np.ndarray:
    global _NC
    if _NC is None:
        _NC = build()

    x = np.ascontiguousarray(x, dtype=np.float32)
    W = np.ascontiguousarray(W, dtype=np.float32)
    bias = np.ascontiguousarray(bias, dtype=np.float32)

    xf = x.reshape(B, RI)
    x8 = (xf * SX).astype(ml_dtypes.float8_e4m3fn)
    # xt8[p, g, b] = x8[b, g*128+p]
    xt8 = np.ascontiguousarray(
        x8.T.reshape(NG, 128, B).transpose(1, 0, 2))
    # xb8[pb, g, kb, col] = x8[kb*128+pb, g*128+col]
    xb8 = np.ascontiguousarray(
        x8.reshape(NB, 128, NG, 128).transpose(1, 2, 0, 3))
    # w16[p, g, (o,c)] = W~[g*128+p, (c,o)] in o-major column order
    wk = W.transpose(0, 3, 1, 2).reshape(RI, C, O)     # [(ri), c, o]
    w_om = wk.reshape(NG, 128, C, O).transpose(1, 0, 3, 2).reshape(
        128, NG, CO)
    w16 = np.ascontiguousarray(w_om.astype(np.float16))
    w8 = np.ascontiguousarray((w_om * SW).astype(ml_dtypes.float8_e4m3fn))
    # bias rows pre-scaled by 1/scal per iteration (o-major)
    bias_om = bias.T.reshape(CO)
    biasf = np.stack([bias_om * (SX * SW / 0.1), bias_om * SX,
                      bias_om]).astype(np.float16)
    sel = np.zeros((128, RPG), dtype=np.float16)
    sel[np.arange(128), np.arange(128) // I] = ISUM
    selT = np.zeros((RPG, 128), dtype=np.float16)
    selT[np.arange(128) // I, np.arange(128)] = 1.0
    xt16 = xf.T.reshape(NG, 128, B).transpose(1, 0, 2).astype(np.float16)

    in_maps = []
    for k in range(N_CORES):
        xo16 = np.ascontiguousarray(
            xt16[:, :, k * B_SHARD:(k + 1) * B_SHARD])
        in_maps.append({
            "xt8": xt8,
            "w8": w8,
            "xb8": xb8,
            "xo16": xo16,
            "w16": w16,
            "biasf": biasf,
            "sel": sel,
            "selT": selT,
        })

    global LAST_RESULT
    res = run_bass_kernel_spmd(
        _NC, in_maps, list(range(N_CORES)),
        trace=bool(os.environ.get("BASS_TRACE")),
    )
    LAST_RESULT = res
    # y columns are o-major: y[b, o*10+c] -> v[b, c, o]
    ys = [res.results[k]["y"].reshape(B_SHARD, O, C).transpose(0, 2, 1)
          for k in range(N_CORES)]
    v = np.concatenate(ys, axis=0)
    return v.reshape(B, C, O)[..., None].astype(np.float32)
